# revision 1
# baseline (speedup 1.0000x reference)
"""VQ codebook kernel (nn_NaiveCodebook) for 8 TRN2 NeuronCores.

Math (per batch row r):
    x   = (img1 - img2) @ W_in                      (b_in cancels in x1-x2)
    d2k = ||x||^2 - 2<x, b_k> + ||b_k||^2
    norm_res = sqrt(min_k d2k)                      (no argmin/gather needed:
                                                     d2[argmin] == min d2)
    scale = norm_res / ||rand|| + eps
    out = (x + scale * rand) @ W_out + b_out

Sharding: data-parallel over the 4096-row batch (512 rows per core);
W_in / book / W_out replicated.  Host-side work is layout only
(transposes / reshapes) plus constant-folding ||b_k||^2/2 from the
codebook weights.

Device pipeline per core (all fp32, matmuls in fp32r mode):
  A: stream img1^T/img2^T/W_in in 1MB batches; diff on DVE; accumulate
     x^T = W_in^T @ diff^T into 4 PSUM banks (contraction 12288).
  B: stream book^T; per 512-code tile matmul G = x^T-slices vs book^T,
     fused  min-reduce  min_k(||b||^2 - 2G)  via tensor_tensor_reduce
     with running-min chaining.
  S: small scalar chain -> scale per row; build quant^T = x^T + s*rand^T.
  C: stream W_out; out tiles = quant^T.T @ W_out + b_out (DVE add), DMA out.
"""

import os
import sys

for _p in (
    "/root/.axon_site",
    "/root/.axon_site/_ro/trn_rl_repo",
    "/opt/trn_rl_repo",
):
    if os.path.isdir(_p) and _p not in sys.path:
        sys.path.append(_p)

import numpy as np

import concourse.bacc as bacc
import concourse.bass as bass
import concourse.tile as tile
from concourse import mybir
from concourse.bass_utils import run_bass_kernel_spmd

F32 = mybir.dt.float32
F32R = mybir.dt.float32r
ALU = mybir.AluOpType

B, C_, H_, W_ = 4096, 3, 64, 64
IN_DIM = C_ * H_ * W_  # 12288
EMB = 512
K = 8192
EPS = 1e-6
NCORES = 8
P = 128
FMAX = 3.0e38


def build_program(rows=B // NCORES, in_dim=IN_DIM, emb=EMB, k=K, kb=4, phases="abc"):
    """Build the single-core Bass program (SPMD across 8 cores).

    Parameterized so a shrunken version can run under CoreSim quickly.
    """
    assert rows % P == 0 and emb % P == 0 and in_dim % (P * kb) == 0
    assert k % 512 == 0 and in_dim % 512 == 0
    mch = rows // P          # row chunks
    ech = emb // P           # emb chunks
    nkb = in_dim // (P * kb)  # phase-A DMA batches
    nd = k // 512            # codebook tiles
    no = in_dim // 512       # output column tiles
    assert no % 4 == 0

    nc = bacc.Bacc()
    img12T = nc.declare_dram_parameter("img12T", [in_dim, 2, rows], F32, isOutput=False)
    w_in = nc.declare_dram_parameter("w_in", [in_dim, emb], F32, isOutput=False)
    bookT = nc.declare_dram_parameter("bookT", [emb, k], F32, isOutput=False)
    c2 = nc.declare_dram_parameter("c2", [1, k], F32, isOutput=False)
    randT = nc.declare_dram_parameter("randT", [emb, rows], F32, isOutput=False)
    w_out = nc.declare_dram_parameter("w_out", [emb, in_dim], F32, isOutput=False)
    b_out = nc.declare_dram_parameter("b_out", [1, in_dim], F32, isOutput=False)
    out = nc.declare_dram_parameter("out", [rows, in_dim], F32, isOutput=True)

    def bcast_ap(handle, count):
        ap = handle.ap()
        return bass.AP(
            tensor=ap.tensor,
            offset=ap.offset,
            ap=[[0, count]] + list(ap.ap)[1:],
        )

    with tile.TileContext(nc) as tc:
        with tc.tile_pool(name="persist", bufs=1) as persist:
            xT = persist.tile([P, ech, rows], F32R, tag="xT")
            rT = persist.tile([P, ech, rows], F32, tag="rT")
            qT = persist.tile([P, ech, rows], F32R, tag="qT")
            m_fin = persist.tile([P, mch], F32, tag="m_fin")
            sxa = persist.tile([P, rows], F32, tag="sxa")
            nra = persist.tile([P, rows], F32, tag="nra")
            mT = persist.tile([1, rows], F32, tag="mT")
            scT = persist.tile([1, rows], F32, tag="scT")
            sc_b = persist.tile([P, rows], F32, tag="sc_b")

            # randT load (small, 1MB) - needed in phase S
            nc.sync.dma_start(
                out=rT, in_=randT.ap().rearrange("(e p) r -> p e r", p=P)
            )

            # ---------------- Phase A: xT = W_in^T @ diff^T ----------------
            i12r = img12T.ap().rearrange("(n j p) t r -> n p j t r", p=P, j=kb)
            wir = w_in.ap().rearrange("(n j p) e -> n p j e", p=P, j=kb)
            with (
                tc.tile_pool(name="astream", bufs=3) as ast,
                tc.tile_pool(name="psum_a", bufs=1, space="PSUM") as psa,
            ):
                px = [psa.tile([P, rows], F32, tag=f"px{e}", name=f"px{e}") for e in range(ech)]
                for n in range(nkb):
                    t12 = ast.tile([P, kb, 2, rows], F32, tag="t12")
                    wt = ast.tile([P, kb, emb], F32R, tag="wi")
                    dt = ast.tile([P, kb, rows], F32R, tag="dt")
                    nc.sync.dma_start(out=t12, in_=i12r[n])
                    nc.sync.dma_start(out=wt, in_=wir[n].bitcast(F32R))
                    nc.vector.tensor_sub(
                        dt, t12[:, :, 0, :], t12[:, :, 1, :]
                    )
                    for j in range(kb):
                        for e in range(ech):
                            nc.tensor.matmul(
                                px[e],
                                lhsT=wt[:, j, e * P : (e + 1) * P],
                                rhs=dt[:, j, :],
                                start=(n == 0 and j == 0),
                                stop=(n == nkb - 1 and j == kb - 1),
                            )
                for e in range(ech):
                    nc.vector.tensor_copy(xT[:, e, :], px[e])

            # -------- Phase B: m = min_k (||b_k||^2 - 2 G) ----------------
            btr = bookT.ap().rearrange("(e p) k -> p e k", p=P)
            with (
                tc.tile_pool(name="bconst", bufs=1) as bconst,
                tc.tile_pool(name="bstream", bufs=3) as bst,
                tc.tile_pool(name="bscratch", bufs=2) as bscr,
                tc.tile_pool(name="bmins", bufs=1) as bmins,
                tc.tile_pool(name="psum_b", bufs=4, space="PSUM") as psb,
            ):
                c2b = bconst.tile([P, k], F32, tag="c2b")
                nc.sync.dma_start(out=c2b, in_=bcast_ap(c2, P))
                mping = [bmins.tile([P, 1], F32, tag=f"mp{m}", name=f"mp{m}") for m in range(mch)]
                mpong = [bmins.tile([P, 1], F32, tag=f"mq{m}", name=f"mq{m}") for m in range(mch)]
                for n in range(nd):
                    bt = bst.tile([P, ech, 512], F32R, tag="bt")
                    nc.sync.dma_start(out=bt, in_=btr[:, :, n * 512 : (n + 1) * 512].bitcast(F32R))
                    for m in range(mch):
                        ps = psb.tile([P, 512], F32, tag="d")
                        for e in range(ech):
                            nc.tensor.matmul(
                                ps,
                                lhsT=xT[:, e, m * P : (m + 1) * P],
                                rhs=bt[:, e, :],
                                start=(e == 0),
                                stop=(e == ech - 1),
                            )
                        # scr = c/2 - G ; tile-min ; running min (m = 2*min)
                        scr = bscr.tile([P, 512], F32, tag="scr")
                        nc.vector.tensor_sub(
                            scr, c2b[:, n * 512 : (n + 1) * 512], ps
                        )
                        tmin = bscr.tile([P, 1], F32, tag="tmin")
                        nc.vector.tensor_reduce(
                            tmin, scr, axis=mybir.AxisListType.X, op=ALU.min
                        )
                        prev = mping[m] if n % 2 == 1 else mpong[m]
                        cur = mpong[m] if n % 2 == 1 else mping[m]
                        if n == 0:
                            nc.vector.tensor_copy(cur[:, 0:1], tmin)
                        elif n == nd - 1:
                            nc.vector.tensor_tensor(
                                m_fin[:, m : m + 1], tmin, prev[:, 0:1], op=ALU.min
                            )
                        else:
                            nc.vector.tensor_tensor(
                                cur[:, 0:1], tmin, prev[:, 0:1], op=ALU.min
                            )

                # ---------- Phase S: per-row scalars + quant^T ----------
                # sxa = sum_e xT^2 (free dim = rows), tree-reduce partitions
                sq = bscr.tile([P, rows], F32, tag="sq")
                nc.vector.tensor_mul(sxa, xT[:, 0, :].bitcast(F32), xT[:, 0, :].bitcast(F32))
                for e in range(1, ech):
                    nc.vector.tensor_mul(sq, xT[:, e, :].bitcast(F32), xT[:, e, :].bitcast(F32))
                    nc.vector.tensor_add(sxa, sxa, sq)
                nc.vector.tensor_mul(nra, rT[:, 0, :], rT[:, 0, :])
                for e in range(1, ech):
                    nc.vector.tensor_mul(sq, rT[:, e, :], rT[:, e, :])
                    nc.vector.tensor_add(nra, nra, sq)
                # cross-partition sums via ones^T matmul (K=128, M=1, N=rows)
                ones128 = bmins.tile([P, 1], F32, tag="ones128")
                nc.vector.memset(ones128, 1.0)
                ps_sx = psb.tile([1, rows], F32, tag="psx", name="ps_sx", bufs=1)
                ps_nr = psb.tile([1, rows], F32, tag="pnr", name="ps_nr", bufs=1)
                nc.tensor.matmul(
                    ps_sx, lhsT=ones128, rhs=sxa,
                    start=True, stop=True,
                )
                nc.tensor.matmul(
                    ps_nr, lhsT=ones128, rhs=nra,
                    start=True, stop=True,
                )
                # mT[0, m*P + p] = m_fin[p, m]  (partition -> free transpose)
                for m in range(mch):
                    nc.sync.dma_start(
                        out=mT[0:1, m * P : (m + 1) * P],
                        in_=m_fin[:, m : m + 1],
                    )
                ns2 = bmins.tile([1, rows], F32, tag="ns2")
                nres = bmins.tile([1, rows], F32, tag="nres")
                nrnd = bmins.tile([1, rows], F32, tag="nrnd")
                rrec = bmins.tile([1, rows], F32, tag="rrec")
                # mT holds min(c/2 - G); ns2 = sx + 2*mT
                mT2 = bmins.tile([1, rows], F32, tag="mT2")
                nc.vector.tensor_scalar_mul(mT2, mT, 2.0)
                nc.vector.tensor_add(ns2, ps_sx[0:1, :], mT2)
                nc.scalar.sqrt(nres, ns2)
                nc.scalar.sqrt(nrnd, ps_nr[0:1, :])
                nc.vector.reciprocal(rrec, nrnd)
                nc.vector.tensor_mul(scT, nres, rrec)
                nc.vector.tensor_scalar_add(scT, scT, EPS)
                # SBUF partition-broadcast isn't a legal DMA AP; bounce via DRAM
                sc_dram = nc.dram_tensor("sc_dram", [1, rows], F32)
                nc.sync.dma_start(out=sc_dram[:, :], in_=scT)
                nc.sync.dma_start(out=sc_b, in_=bcast_ap(sc_dram, P))
                tmp = bscr.tile([P, rows], F32, tag="tmp")
                for e in range(ech):
                    nc.vector.tensor_mul(tmp, rT[:, e, :], sc_b)
                    nc.vector.tensor_add(qT[:, e, :], xT[:, e, :].bitcast(F32), tmp)

            # -------- Phase C: out = quant @ W_out + b_out ----------------
            wor = w_out.ap().rearrange("(e p) d -> p e d", p=P)
            outap = out.ap()
            with (
                tc.tile_pool(name="cconst", bufs=1) as cconst,
                tc.tile_pool(name="cstream", bufs=3) as cst,
                tc.tile_pool(name="couts", bufs=2) as cout,
                tc.tile_pool(name="psum_c", bufs=4, space="PSUM") as psc,
            ):
                bb = cconst.tile([P, in_dim], F32, tag="bb")
                nc.sync.dma_start(out=bb, in_=bcast_ap(b_out, P))
                for g in range(no // 4):
                    osb = [
                        cout.tile([P, 4, 512], F32, tag=f"osb{m}", name=f"osb{m}") for m in range(mch)
                    ]
                    for nin in range(4):
                        n = g * 4 + nin
                        wt = cst.tile([P, ech, 512], F32R, tag="wo")
                        nc.sync.dma_start(
                            out=wt, in_=wor[:, :, n * 512 : (n + 1) * 512].bitcast(F32R)
                        )
                        for m in range(mch):
                            ps = psc.tile([P, 512], F32, tag="o")
                            for e in range(ech):
                                nc.tensor.matmul(
                                    ps,
                                    lhsT=qT[:, e, m * P : (m + 1) * P],
                                    rhs=wt[:, e, :],
                                    start=(e == 0),
                                    stop=(e == ech - 1),
                                )
                            nc.vector.tensor_add(
                                osb[m][:, nin, :],
                                ps,
                                bb[:, n * 512 : (n + 1) * 512],
                            )
                    for m in range(mch):
                        nc.sync.dma_start(
                            out=outap[
                                m * P : (m + 1) * P, g * 2048 : (g + 1) * 2048
                            ],
                            in_=osb[m],
                        )
    nc.finalize()
    return nc


def make_shards(image_1, image_2, random_vector, W_in, b_in, W_out, b_out, book,
                rows=B // NCORES, ncores=NCORES):
    x1 = np.ascontiguousarray(np.asarray(image_1, np.float32)).reshape(-1, IN_DIM if image_1.ndim == 4 else image_1.shape[-1])
    # generic reshape: flatten all dims after the first
    x1 = np.asarray(image_1, np.float32).reshape(image_1.shape[0], -1)
    x2 = np.asarray(image_2, np.float32).reshape(image_2.shape[0], -1)
    rv = np.asarray(random_vector, np.float32)
    in_dim = x1.shape[1]
    emb = W_in.shape[1]
    k = book.shape[0]
    w_in_c = np.ascontiguousarray(np.asarray(W_in, np.float32))
    bookT_c = np.ascontiguousarray(np.asarray(book, np.float32).T)
    c2_c = (np.sum(np.asarray(book, np.float64) ** 2, axis=1) / 2.0).astype(
        np.float32
    ).reshape(1, k)
    w_out_c = np.ascontiguousarray(np.asarray(W_out, np.float32))
    b_out_c = np.ascontiguousarray(np.asarray(b_out, np.float32)).reshape(1, in_dim)
    shards = []
    for i in range(ncores):
        sl = slice(i * rows, (i + 1) * rows)
        img12 = np.empty((x1.shape[1], 2, rows), np.float32)
        img12[:, 0, :] = x1[sl].T
        img12[:, 1, :] = x2[sl].T
        shards.append(
            {
                "img12T": img12,
                "w_in": w_in_c,
                "bookT": bookT_c,
                "c2": c2_c,
                "randT": np.ascontiguousarray(rv[sl].T),
                "w_out": w_out_c,
                "b_out": b_out_c,
            }
        )
    return shards


_prog_cache = {}


def _get_program():
    if "nc" not in _prog_cache:
        _prog_cache["nc"] = build_program()
    return _prog_cache["nc"]


def run(inputs, trace=False):
    """Run on the 8 NeuronCores; returns (full_output, BassKernelResults)."""
    nc = _get_program()
    shards = make_shards(**inputs)
    res = run_bass_kernel_spmd(nc, shards, core_ids=list(range(NCORES)), trace=trace)
    out = np.concatenate([res.results[i]["out"] for i in range(NCORES)], axis=0)
    return out, res


def kernel(**inputs):
    out, _ = run(inputs, trace=False)
    return out



# revision 6
# speedup vs baseline: 1.4293x; 1.4293x over previous
"""VQ codebook kernel (nn_NaiveCodebook) for 8 TRN2 NeuronCores — bf16 v2.

Math (per batch row r):
    x   = (img1 - img2) @ W_in                      (b_in cancels in x1-x2)
    d2k = ||x||^2 - 2<x, b_k> + ||b_k||^2
    norm_res = sqrt(min_k d2k)
    scale = norm_res / ||rand|| + eps
    out = (x + scale * rand) @ W_out + b_out

All HBM streams are bf16 (host pre-casts); matmuls bf16 (fp32 PSUM).
Host-side work is layout only (transpose / cast) plus constant-folding
||b_k||^2/2 from the codebook weights.

Device pipeline per core (rows = 512):
  A: stream img1^T/img2^T (interleaved) + W_in in bf16; diff on DVE;
     accumulate x^T = W_in^T @ diff^T into 4 PSUM banks (K=12288).
  B: stream book^T; per 512-code tile G = x^T-chunks @ book-tile; fused
     tensor_tensor_reduce: out=(c2/2 - G)*2, running-min chain into
     m_all (min_k d2 = ||x||^2 + m).
  S: ||x||^2 / ||rand||^2 via ACT Square + ones-matmul partition sums;
     scale = sqrt(ns2) * rsqrt(nr) + eps; broadcast scale to 128
     partitions with a K=1 ones-matmul outer product; q^T = x^T + s*r^T.
  C: stream W_out; bias folded into PSUM via K=1 ones (x) b_out matmul;
     out tiles evacuated as bf16 (DVE/ACT alternating), DMA out.
"""

import os
import sys

for _p in (
    "/root/.axon_site",
    "/root/.axon_site/_ro/trn_rl_repo",
    "/opt/trn_rl_repo",
):
    if os.path.isdir(_p) and _p not in sys.path:
        sys.path.append(_p)

import numpy as np
import ml_dtypes

import concourse.bacc as bacc
import concourse.bass as bass
import concourse.tile as tile
from concourse import mybir
from concourse.bass_utils import run_bass_kernel_spmd

F32 = mybir.dt.float32
F32R = mybir.dt.float32r
BF16 = mybir.dt.bfloat16
ALU = mybir.AluOpType
ACTF = mybir.ActivationFunctionType
BF = ml_dtypes.bfloat16

B, C_, H_, W_ = 4096, 3, 64, 64
IN_DIM = C_ * H_ * W_  # 12288
EMB = 512
K = 8192
EPS = 1e-6
NCORES = 8
P = 128
FMAX = 3.0e38


def build_program(rows=B // NCORES, in_dim=IN_DIM, emb=EMB, k=K, kb=8):
    """Single-core Bass program (SPMD across 8 cores)."""
    assert rows % P == 0 and emb % P == 0 and in_dim % (P * kb) == 0
    assert k % 1024 == 0 and in_dim % 1024 == 0
    mch = rows // P           # row chunks (4)
    ech = emb // P            # emb chunks (4)
    nkb = in_dim // (P * kb)  # phase-A DMA batches (12)
    nd = k // 512             # codebook 512-tiles (16)
    ndp = nd // 2             # paired book DMAs (8)
    no = in_dim // 512        # output column tiles (24)
    nop = no // 2             # paired w_out DMAs (12)

    nc = bacc.Bacc()
    img12T = nc.declare_dram_parameter("img12T", [in_dim, 2, rows], BF16, isOutput=False)
    w_in = nc.declare_dram_parameter("w_in", [in_dim, emb], BF16, isOutput=False)
    bookT = nc.declare_dram_parameter("bookT", [emb, k], BF16, isOutput=False)
    c2 = nc.declare_dram_parameter("c2", [1, k], F32, isOutput=False)
    randT = nc.declare_dram_parameter("randT", [emb, rows], BF16, isOutput=False)
    w_out = nc.declare_dram_parameter("w_out", [emb, in_dim], BF16, isOutput=False)
    b_out = nc.declare_dram_parameter("b_out", [1, in_dim], BF16, isOutput=False)
    out = nc.declare_dram_parameter("out", [rows, in_dim], BF16, isOutput=True)

    def bcast_ap(handle, count):
        ap = handle.ap()
        return bass.AP(
            tensor=ap.tensor,
            offset=ap.offset,
            ap=[[0, count]] + list(ap.ap)[1:],
        )

    with tile.TileContext(nc) as tc:
        with tc.tile_pool(name="persist", bufs=1) as persist:
            xT = persist.tile([P, ech, rows], BF16, tag="xT")
            rT = persist.tile([P, ech, rows], BF16, tag="rT")
            qT = persist.tile([P, ech, rows], BF16, tag="qT")
            m_all = [
                persist.tile([P, nd], F32, tag=f"ma{m}", name=f"ma{m}")
                for m in range(mch)
            ]
            m_fin = [
                persist.tile([P, 1], F32, tag=f"mf{m}", name=f"mf{m}")
                for m in range(mch)
            ]
            ones_k = persist.tile([P, 1], BF16, tag="ones_k")    # K=128 partition-sum lhsT
            ones_m = persist.tile([1, P], BF16, tag="ones_m")    # K=1 fold lhsT
            mT = persist.tile([1, rows], F32, tag="mT")
            ns2 = persist.tile([1, rows], F32, tag="ns2")
            nres = persist.tile([1, rows], F32, tag="nres")
            nrir = persist.tile([1, rows], F32, tag="nrir")
            scT = persist.tile([1, rows], F32, tag="scT")

            nc.vector.memset(ones_k, 1.0)
            nc.vector.memset(ones_m, 1.0)
            nc.sync.dma_start(
                out=rT, in_=randT.ap().rearrange("(e p) r -> p e r", p=P)
            )

            # ---------------- Phase A: xT = W_in^T @ diff^T ----------------
            i12r = img12T.ap().rearrange("(n j p) t r -> n p j t r", p=P, j=kb)
            wir = w_in.ap().rearrange("(n j p) e -> n p j e", p=P, j=kb)
            with (
                tc.tile_pool(name="astream", bufs=2) as ast,
                tc.tile_pool(name="psum_a", bufs=1, space="PSUM") as psa,
            ):
                px = [
                    psa.tile([P, rows], F32, tag=f"px{e}", name=f"px{e}")
                    for e in range(ech)
                ]
                for n in range(nkb):
                    t12 = ast.tile([P, kb, 2, rows], BF16, tag="t12")
                    wt = ast.tile([P, kb, emb], BF16, tag="wi")
                    dt = ast.tile([P, kb, rows], BF16, tag="dt")
                    nc.sync.dma_start(out=t12, in_=i12r[n])
                    nc.sync.dma_start(out=wt, in_=wir[n])
                    nc.vector.tensor_sub(dt, t12[:, :, 0, :], t12[:, :, 1, :])
                    for j in range(kb):
                        for e in range(ech):
                            nc.tensor.matmul(
                                px[e],
                                lhsT=wt[:, j, e * P : (e + 1) * P],
                                rhs=dt[:, j, :],
                                start=(n == 0 and j == 0),
                                stop=(n == nkb - 1 and j == kb - 1),
                            )
                for e in range(ech):
                    nc.vector.tensor_copy(xT[:, e, :], px[e])

            # -------- Phase B: running max_k (G - c2/2) -------------------
            # c2/2 is folded into PSUM with a K=1 ones (x) (-c2/2) matmul;
            # one DVE tensor_reduce(max) per 512-code tile writes a column
            # of m_all; a single final reduce collapses the nd columns.
            btr = bookT.ap().rearrange("(e p) (b tn) -> b p e tn", p=P, tn=1024)
            with (
                tc.tile_pool(name="bconst", bufs=1) as bconst,
                tc.tile_pool(name="bstream", bufs=3) as bst,
                tc.tile_pool(name="bscratch", bufs=2) as bscr,
                tc.tile_pool(name="psum_b", bufs=4, space="PSUM") as psb,
                tc.tile_pool(name="psum_s", bufs=1, space="PSUM") as pss,
            ):
                c2n = bconst.tile([1, k], BF16, tag="c2n")
                nc.gpsimd.dma_start(out=c2n, in_=c2.ap())
                nc.vector.tensor_scalar_mul(c2n, c2n, -1.0)
                for bp in range(ndp):
                    bt = bst.tile([P, ech, 1024], BF16, tag="bt")
                    nc.sync.dma_start(out=bt, in_=btr[bp])
                    for t in range(2):
                        n = bp * 2 + t
                        for m in range(mch):
                            ps = psb.tile([P, 512], F32, tag="d")
                            nc.tensor.matmul(
                                ps,
                                lhsT=ones_m,
                                rhs=c2n[0:1, n * 512 : (n + 1) * 512],
                                start=True,
                                stop=False,
                            )
                            for e in range(ech):
                                nc.tensor.matmul(
                                    ps,
                                    lhsT=xT[:, e, m * P : (m + 1) * P],
                                    rhs=bt[:, e, t * 512 : (t + 1) * 512],
                                    start=False,
                                    stop=(e == ech - 1),
                                )
                            nc.vector.tensor_reduce(
                                m_all[m][:, n : n + 1],
                                ps,
                                axis=mybir.AxisListType.X,
                                op=ALU.max,
                            )
                for m in range(mch):
                    nc.vector.tensor_reduce(
                        m_fin[m],
                        m_all[m],
                        axis=mybir.AxisListType.X,
                        op=ALU.max,
                    )

                # ---------- Phase S: per-row scalars + quant^T ----------
                # sum_emb x^2 and rand^2: ACT Square + ones-matmul partition sums
                ps_sx = pss.tile([1, rows], F32, tag="psx", name="ps_sx")
                ps_nr = pss.tile([1, rows], F32, tag="pnr", name="ps_nr")
                for src_t, ps_dst in ((xT, ps_sx), (rT, ps_nr)):
                    for e in range(ech):
                        sq = bscr.tile([P, rows], BF16, tag="sq")
                        nc.scalar.activation(sq, src_t[:, e, :], ACTF.Square)
                        nc.tensor.matmul(
                            ps_dst,
                            lhsT=ones_k,
                            rhs=sq,
                            start=(e == 0),
                            stop=(e == ech - 1),
                        )
                # mT[0, m*P + p] = m_all[m][p, nd-1]  (partition -> free)
                for m in range(mch):
                    nc.sync.dma_start(
                        out=mT[0:1, m * P : (m + 1) * P],
                        in_=m_fin[m],
                    )
                # ns2 = ||x||^2 - 2 max(G - c2/2) = min_k d2
                nc.vector.tensor_scalar_mul(mT, mT, -2.0)
                nc.vector.tensor_add(ns2, ps_sx[0:1, :], mT)
                nc.scalar.sqrt(nres, ns2)
                nrnd = bscr.tile([1, rows], F32, tag="nrnd")
                nc.scalar.sqrt(nrnd, ps_nr[0:1, :])
                nc.vector.reciprocal(nrir, nrnd)
                nc.vector.tensor_mul(scT, nres, nrir)
                nc.vector.tensor_scalar_add(scT, scT, EPS)
                # broadcast scT to all partitions via DRAM bounce
                sc_dram = nc.dram_tensor("sc_dram", [1, rows], F32)
                sc_b = bconst.tile([P, rows], F32, tag="sc_b")
                nc.sync.dma_start(out=sc_dram[:, :], in_=scT)
                nc.sync.dma_start(out=sc_b, in_=bcast_ap(sc_dram, P))
                tmp = bscr.tile([P, rows], F32, tag="tmp")
                for e in range(ech):
                    nc.vector.tensor_mul(tmp, rT[:, e, :], sc_b)
                    nc.vector.tensor_add(qT[:, e, :], xT[:, e, :], tmp)

            # -------- Phase C: out = quant @ W_out + b_out ----------------
            wor = w_out.ap().rearrange("(e p) (g tn) -> g p e tn", p=P, tn=1024)
            outap = out.ap()
            with (
                tc.tile_pool(name="cconst", bufs=1) as cconst,
                tc.tile_pool(name="cstream", bufs=3) as cst,
                tc.tile_pool(name="couts", bufs=2) as cout,
                tc.tile_pool(name="psum_c", bufs=4, space="PSUM") as psc,
            ):
                bbb = cconst.tile([P, in_dim], BF16, tag="bbb")
                nc.sync.dma_start(out=bbb, in_=bcast_ap(b_out, P))
                osb = None
                for gg in range(nop):
                    wt = cst.tile([P, ech, 1024], BF16, tag="wo")
                    nc.sync.dma_start(out=wt, in_=wor[gg])
                    for t in range(2):
                        n = gg * 2 + t
                        if n % 4 == 0:
                            osb = [
                                cout.tile([P, 4, 512], BF16, tag=f"osb{m}", name=f"osb{m}")
                                for m in range(mch)
                            ]
                        for m in range(mch):
                            ps = psc.tile([P, 512], F32, tag="o")
                            for e in range(ech):
                                nc.tensor.matmul(
                                    ps,
                                    lhsT=qT[:, e, m * P : (m + 1) * P],
                                    rhs=wt[:, e, t * 512 : (t + 1) * 512],
                                    start=(e == 0),
                                    stop=(e == ech - 1),
                                )
                            nc.vector.tensor_add(
                                osb[m][:, n % 4, :],
                                ps,
                                bbb[:, n * 512 : (n + 1) * 512],
                            )
                        if n % 4 == 3:
                            g = n // 4
                            for m in range(mch):
                                nc.sync.dma_start(
                                    out=outap[
                                        m * P : (m + 1) * P,
                                        g * 2048 : (g + 1) * 2048,
                                    ],
                                    in_=osb[m],
                                )
    nc.finalize()
    return nc


def make_shards(image_1, image_2, random_vector, W_in, b_in, W_out, b_out, book,
                rows=None, ncores=NCORES):
    x1 = np.asarray(image_1, np.float32).reshape(np.shape(image_1)[0], -1)
    x2 = np.asarray(image_2, np.float32).reshape(np.shape(image_2)[0], -1)
    rv = np.asarray(random_vector, np.float32)
    nrows_total = x1.shape[0]
    if rows is None:
        rows = nrows_total // ncores
    in_dim = x1.shape[1]
    k = np.shape(book)[0]
    x1b = x1.astype(BF)
    x2b = x2.astype(BF)
    w_in_c = np.ascontiguousarray(np.asarray(W_in, np.float32).astype(BF))
    bookT_c = np.ascontiguousarray(np.asarray(book, np.float32).astype(BF).T)
    c2_c = (np.sum(np.asarray(book, np.float64) ** 2, axis=1) / 2.0).astype(
        np.float32
    ).reshape(1, k)
    w_out_c = np.ascontiguousarray(np.asarray(W_out, np.float32).astype(BF))
    b_out_c = np.ascontiguousarray(
        np.asarray(b_out, np.float32).astype(BF)
    ).reshape(1, in_dim)
    shards = []
    for i in range(ncores):
        sl = slice(i * rows, (i + 1) * rows)
        img12 = np.empty((in_dim, 2, rows), BF)
        img12[:, 0, :] = x1b[sl].T
        img12[:, 1, :] = x2b[sl].T
        shards.append(
            {
                "img12T": img12,
                "w_in": w_in_c,
                "bookT": bookT_c,
                "c2": c2_c,
                "randT": np.ascontiguousarray(rv[sl].T.astype(BF)),
                "w_out": w_out_c,
                "b_out": b_out_c,
            }
        )
    return shards


_prog_cache = {}


def _get_program():
    if "nc" not in _prog_cache:
        _prog_cache["nc"] = build_program()
    return _prog_cache["nc"]


def run(inputs, trace=False):
    """Run on the 8 NeuronCores; returns (full_output, BassKernelResults)."""
    nc = _get_program()
    shards = make_shards(**inputs)
    res = run_bass_kernel_spmd(nc, shards, core_ids=list(range(NCORES)), trace=trace)
    out = np.concatenate(
        [np.asarray(res.results[i]["out"], np.float32) for i in range(NCORES)],
        axis=0,
    )
    return out, res


def kernel(**inputs):
    out, _ = run(inputs, trace=False)
    return out


# revision 7
# speedup vs baseline: 1.5354x; 1.0742x over previous
"""VQ codebook kernel (nn_NaiveCodebook) for 8 TRN2 NeuronCores — bf16 v5.

Math (per batch row r):
    x   = (img1 - img2) @ W_in                      (b_in cancels in x1-x2)
    d2k = ||x||^2 - 2<x, b_k> + ||b_k||^2
    norm_res = sqrt(min_k d2k)
    scale = norm_res / ||rand|| + eps
    out = (x + scale * rand) @ W_out + b_out

All HBM streams are bf16, host pre-tiled so each DMA is 128 partitions
x one contiguous run.  Host-side work is layout only (transpose / cast /
tiling) plus constant-folding ||b_k||^2/2 from the codebook weights.

Device pipeline per core (rows = 512):
  A: stream img1^T/img2^T (interleaved) + W_in; diff on DVE; accumulate
     x^T = W_in^T @ diff^T into 4 PSUM banks (K=12288).  ||rand||^2 and
     its sqrt/reciprocal chain run here too (they have no deps).
  B: stream book^T; per 512-code tile: -c2/2 seeded into PSUM via a K=1
     ones (x) (-c2/2) matmul, then G accumulates; one DVE
     tensor_reduce(max) per tile into a column of m_all; one final
     reduce collapses the columns.
  S: ||x||^2 via ACT Square + ones-matmul partition sums; ns2 =
     -2*max + ||x||^2 (one fused op); scale = sqrt(ns2)/||rand|| + eps;
     broadcast via K=1 ones (x) scale matmul; q^T = x^T + s*r^T.
  C: stream W_out; paired 2-bank PSUM tiles; one DVE add (+bias bcast
     tile) per 1024 columns writes bf16 out tiles; DMA out.
"""

import os
import sys

for _p in (
    "/root/.axon_site",
    "/root/.axon_site/_ro/trn_rl_repo",
    "/opt/trn_rl_repo",
):
    if os.path.isdir(_p) and _p not in sys.path:
        sys.path.append(_p)

import numpy as np
import ml_dtypes

import concourse.bacc as bacc
import concourse.bass as bass
import concourse.tile as tile
from concourse import mybir
from concourse.bass_utils import run_bass_kernel_spmd

F32 = mybir.dt.float32
BF16 = mybir.dt.bfloat16
ALU = mybir.AluOpType
ACTF = mybir.ActivationFunctionType
BF = ml_dtypes.bfloat16

B, C_, H_, W_ = 4096, 3, 64, 64
IN_DIM = C_ * H_ * W_  # 12288
EMB = 512
K = 8192
EPS = 1e-6
NCORES = 8
P = 128
KB = 8


def build_program(rows=B // NCORES, in_dim=IN_DIM, emb=EMB, k=K, kb=KB):
    """Single-core Bass program (SPMD across 8 cores)."""
    assert rows % P == 0 and emb % P == 0 and in_dim % (P * kb) == 0
    assert k % 1024 == 0 and in_dim % 1024 == 0
    mch = rows // P           # row chunks (4)
    ech = emb // P            # emb chunks (4)
    nkb = in_dim // (P * kb)  # phase-A DMA batches (12)
    nd = k // 512             # codebook 512-tiles (16)
    ndp = nd // 2             # paired book DMAs (8)
    no = in_dim // 512        # output column tiles (24)
    nop = no // 2             # paired w_out DMAs / paired PSUM tiles (12)

    nc = bacc.Bacc()
    img12T = nc.declare_dram_parameter(
        "img12T", [nkb, P, kb, 2, rows], BF16, isOutput=False)
    w_in = nc.declare_dram_parameter(
        "w_in", [nkb, P, kb, emb], BF16, isOutput=False)
    bookT = nc.declare_dram_parameter(
        "bookT", [ndp, P, ech, 1024], BF16, isOutput=False)
    c2 = nc.declare_dram_parameter("c2", [1, k], F32, isOutput=False)
    randT = nc.declare_dram_parameter("randT", [P, ech, rows], BF16, isOutput=False)
    w_out = nc.declare_dram_parameter(
        "w_out", [nop, P, ech, 1024], BF16, isOutput=False)
    b_out = nc.declare_dram_parameter("b_out", [1, in_dim], BF16, isOutput=False)
    out = nc.declare_dram_parameter("out", [rows, in_dim], BF16, isOutput=True)

    def bcast_ap(handle, count):
        ap = handle.ap()
        return bass.AP(
            tensor=ap.tensor,
            offset=ap.offset,
            ap=[[0, count]] + list(ap.ap)[1:],
        )

    with tile.TileContext(nc) as tc:
        with (
            tc.tile_pool(name="persist", bufs=1) as persist,
            tc.tile_pool(name="psum_s", bufs=1, space="PSUM") as pss,
        ):
            xT = persist.tile([P, ech, rows], BF16, tag="xT")
            rT = persist.tile([P, ech, rows], BF16, tag="rT")
            qT = persist.tile([P, ech, rows], BF16, tag="qT")
            m_all = [
                persist.tile([P, nd], F32, tag=f"ma{m}", name=f"ma{m}")
                for m in range(mch)
            ]
            m_fin = [
                persist.tile([P, 1], F32, tag=f"mf{m}", name=f"mf{m}")
                for m in range(mch)
            ]
            ones_k = persist.tile([P, 1], BF16, tag="ones_k")   # K=128 sum lhsT
            ones_m = persist.tile([1, P], BF16, tag="ones_m")   # K=1 fold/bcast lhsT
            c2n = persist.tile([1, k], BF16, tag="c2n")         # -||b||^2/2
            bbb = persist.tile([P, in_dim], BF16, tag="bbb")    # bias bcast
            mT = persist.tile([1, rows], F32, tag="mT")
            ns2 = persist.tile([1, rows], F32, tag="ns2")
            nres = persist.tile([1, rows], F32, tag="nres")
            nrnd = persist.tile([1, rows], F32, tag="nrnd")
            nrir = persist.tile([1, rows], F32, tag="nrir")
            scT = persist.tile([1, rows], F32, tag="scT")
            scb = persist.tile([1, rows], BF16, tag="scb")
            sq = [
                persist.tile([P, rows], BF16, tag=f"sq{i}", name=f"sq{i}")
                for i in range(2)
            ]
            ps_sx = pss.tile([1, rows], F32, tag="psx", name="ps_sx")
            ps_nr = pss.tile([1, rows], F32, tag="pnr", name="ps_nr")
            ps_sc = pss.tile([P, rows], F32, tag="psc", name="ps_sc")

            nc.vector.memset(ones_k, 1.0)
            nc.vector.memset(ones_m, 1.0)
            nc.gpsimd.dma_start(out=c2n, in_=c2.ap())   # f32 -> bf16 cast DMA
            nc.vector.tensor_scalar_mul(c2n, c2n, -1.0)
            nc.sync.dma_start(out=bbb, in_=bcast_ap(b_out, P))
            nc.sync.dma_start(out=rT, in_=randT.ap())

            # ||rand||^2 chain — no deps on anything else; hides under A
            for e in range(ech):
                nc.scalar.activation(sq[e % 2], rT[:, e, :], ACTF.Square)
                nc.tensor.matmul(
                    ps_nr, lhsT=ones_k, rhs=sq[e % 2],
                    start=(e == 0), stop=(e == ech - 1),
                )
            nc.scalar.sqrt(nrnd, ps_nr[0:1, :])
            nc.vector.reciprocal(nrir, nrnd)

            # ---------------- Phase A: xT = W_in^T @ diff^T ----------------
            with (
                tc.tile_pool(name="astream", bufs=2) as ast,
                tc.tile_pool(name="psum_a", bufs=1, space="PSUM") as psa,
            ):
                px = [
                    psa.tile([P, rows], F32, tag=f"px{e}", name=f"px{e}")
                    for e in range(ech)
                ]
                for n in range(nkb):
                    t12 = ast.tile([P, kb, 2, rows], BF16, tag="t12")
                    wt = ast.tile([P, kb, emb], BF16, tag="wi")
                    dt = ast.tile([P, kb, rows], BF16, tag="dt")
                    nc.sync.dma_start(out=t12, in_=img12T.ap()[n])
                    nc.sync.dma_start(out=wt, in_=w_in.ap()[n])
                    nc.vector.tensor_sub(dt, t12[:, :, 0, :], t12[:, :, 1, :])
                    for j in range(kb):
                        for e in range(ech):
                            nc.tensor.matmul(
                                px[e],
                                lhsT=wt[:, j, e * P : (e + 1) * P],
                                rhs=dt[:, j, :],
                                start=(n == 0 and j == 0),
                                stop=(n == nkb - 1 and j == kb - 1),
                            )
                for e in range(ech):
                    if e % 2 == 0:
                        nc.vector.tensor_copy(xT[:, e, :], px[e])
                    else:
                        nc.scalar.copy(xT[:, e, :], px[e])

            # -------- Phase B: running max_k (G - c2/2) -------------------
            with (
                tc.tile_pool(name="bstream", bufs=3) as bst,
                tc.tile_pool(name="psum_b", bufs=4, space="PSUM") as psb,
            ):
                for bp in range(ndp):
                    bt = bst.tile([P, ech, 1024], BF16, tag="bt")
                    nc.sync.dma_start(out=bt, in_=bookT.ap()[bp])
                    for t in range(2):
                        n = bp * 2 + t
                        pss_n = [
                            psb.tile([P, 512], F32, tag="d", name=f"d{n}_{m}")
                            for m in range(mch)
                        ]
                        for m in range(mch):
                            nc.tensor.matmul(
                                pss_n[m],
                                lhsT=ones_m,
                                rhs=c2n[0:1, n * 512 : (n + 1) * 512],
                                start=True,
                                stop=False,
                            )
                        for e in range(ech):
                            for m in range(mch):
                                nc.tensor.matmul(
                                    pss_n[m],
                                    lhsT=xT[:, e, m * P : (m + 1) * P],
                                    rhs=bt[:, e, t * 512 : (t + 1) * 512],
                                    start=False,
                                    stop=(e == ech - 1),
                                )
                        for m in range(mch):
                            nc.vector.tensor_reduce(
                                m_all[m][:, n : n + 1],
                                pss_n[m],
                                axis=mybir.AxisListType.X,
                                op=ALU.max,
                            )
                for m in range(mch):
                    nc.vector.tensor_reduce(
                        m_fin[m], m_all[m], axis=mybir.AxisListType.X, op=ALU.max
                    )

                # ---------- Phase S: per-row scalars + quant^T ----------
                for e in range(ech):
                    nc.scalar.activation(sq[e % 2], xT[:, e, :], ACTF.Square)
                    nc.tensor.matmul(
                        ps_sx, lhsT=ones_k, rhs=sq[e % 2],
                        start=(e == 0), stop=(e == ech - 1),
                    )
                # mT[0, m*P + p] = m_fin[m][p]  (partition -> free)
                for m in range(mch):
                    nc.sync.dma_start(
                        out=mT[0:1, m * P : (m + 1) * P], in_=m_fin[m]
                    )
                # ns2 = min_k d2 = ||x||^2 - 2*max;  scale = sqrt(ns2)/nr + eps
                nc.vector.scalar_tensor_tensor(
                    out=ns2, in0=mT, scalar=-2.0, in1=ps_sx[0:1, :],
                    op0=ALU.mult, op1=ALU.add,
                )
                nc.scalar.sqrt(nres, ns2)
                nc.vector.tensor_mul(scT, nres, nrir)
                nc.vector.tensor_scalar_add(scT, scT, EPS)
                nc.vector.tensor_copy(scb, scT)
                # broadcast: ps_sc = ones (x) scale
                nc.tensor.matmul(
                    ps_sc, lhsT=ones_m, rhs=scb, start=True, stop=True
                )
                for e in range(ech):
                    tmp = bst.tile([P, rows], F32, tag="tmp")
                    nc.vector.tensor_mul(tmp, rT[:, e, :], ps_sc)
                    nc.vector.tensor_add(qT[:, e, :], xT[:, e, :], tmp)

            # -------- Phase C: out = quant @ W_out + b_out ----------------
            outap = out.ap()
            with (
                tc.tile_pool(name="cstream", bufs=3) as cst,
                tc.tile_pool(name="couts", bufs=2) as cout,
                tc.tile_pool(name="psum_c", bufs=2, space="PSUM") as psc,
            ):
                osb = None
                for gg in range(nop):
                    wt = cst.tile([P, ech, 1024], BF16, tag="wo")
                    nc.sync.dma_start(out=wt, in_=w_out.ap()[gg])
                    if gg % 2 == 0:
                        osb = [
                            cout.tile([P, 2, 1024], BF16, tag=f"osb{m}", name=f"osb{m}")
                            for m in range(mch)
                        ]
                    for m in range(mch):
                        ps2 = psc.tile([P, 1024], F32, tag="o")
                        for t in range(2):
                            for e in range(ech):
                                nc.tensor.matmul(
                                    ps2[:, t * 512 : (t + 1) * 512],
                                    lhsT=qT[:, e, m * P : (m + 1) * P],
                                    rhs=wt[:, e, t * 512 : (t + 1) * 512],
                                    start=(e == 0),
                                    stop=(e == ech - 1),
                                )
                        nc.vector.tensor_add(
                            osb[m][:, gg % 2, :],
                            ps2,
                            bbb[:, gg * 1024 : (gg + 1) * 1024],
                        )
                    if gg % 2 == 1:
                        g = gg // 2
                        for m in range(mch):
                            nc.sync.dma_start(
                                out=outap[
                                    m * P : (m + 1) * P,
                                    g * 2048 : (g + 1) * 2048,
                                ],
                                in_=osb[m],
                            )
    nc.finalize()
    return nc


def make_shards(image_1, image_2, random_vector, W_in, b_in, W_out, b_out, book,
                rows=None, ncores=NCORES, kb=KB):
    x1 = np.asarray(image_1, np.float32).reshape(np.shape(image_1)[0], -1)
    x2 = np.asarray(image_2, np.float32).reshape(np.shape(image_2)[0], -1)
    rv = np.asarray(random_vector, np.float32)
    nrows_total = x1.shape[0]
    if rows is None:
        rows = nrows_total // ncores
    in_dim = x1.shape[1]
    emb = np.shape(W_in)[1]
    k = np.shape(book)[0]
    ech = emb // P
    nkb = in_dim // (P * kb)
    ndp = k // 1024
    nop = in_dim // 1024

    x1b = x1.astype(BF)
    x2b = x2.astype(BF)
    w_in_b = np.asarray(W_in, np.float32).astype(BF)
    # w_in tiled: [nkb, P, kb, emb];  row index d = (n*kb + j)*P + p
    w_in_t = np.ascontiguousarray(
        w_in_b.reshape(nkb, kb, P, emb).transpose(0, 2, 1, 3)
    )
    bookT_b = np.asarray(book, np.float32).astype(BF).T  # [emb, k]
    bookT_t = np.ascontiguousarray(
        bookT_b.reshape(ech, P, ndp, 1024).transpose(2, 1, 0, 3)
    )
    c2_c = (np.sum(np.asarray(book, np.float64) ** 2, axis=1) / 2.0).astype(
        np.float32
    ).reshape(1, k)
    w_out_b = np.asarray(W_out, np.float32).astype(BF)  # [emb, in_dim]
    w_out_t = np.ascontiguousarray(
        w_out_b.reshape(ech, P, nop, 1024).transpose(2, 1, 0, 3)
    )
    b_out_c = np.ascontiguousarray(
        np.asarray(b_out, np.float32).astype(BF)
    ).reshape(1, in_dim)
    shards = []
    for i in range(ncores):
        sl = slice(i * rows, (i + 1) * rows)
        # img12 tiled: [nkb, P, kb, 2, rows]; d = (n*kb + j)*P + p
        img12 = np.empty((nkb, P, kb, 2, rows), BF)
        img12[:, :, :, 0, :] = (
            x1b[sl].T.reshape(nkb, kb, P, rows).transpose(0, 2, 1, 3)
        )
        img12[:, :, :, 1, :] = (
            x2b[sl].T.reshape(nkb, kb, P, rows).transpose(0, 2, 1, 3)
        )
        randT_t = np.ascontiguousarray(
            rv[sl].T.astype(BF).reshape(ech, P, rows).transpose(1, 0, 2)
        )
        shards.append(
            {
                "img12T": img12,
                "w_in": w_in_t,
                "bookT": bookT_t,
                "c2": c2_c,
                "randT": randT_t,
                "w_out": w_out_t,
                "b_out": b_out_c,
            }
        )
    return shards


_prog_cache = {}


def _get_program():
    if "nc" not in _prog_cache:
        _prog_cache["nc"] = build_program()
    return _prog_cache["nc"]


def run(inputs, trace=False):
    """Run on the 8 NeuronCores; returns (full_output, BassKernelResults)."""
    nc = _get_program()
    shards = make_shards(**inputs)
    res = run_bass_kernel_spmd(nc, shards, core_ids=list(range(NCORES)), trace=trace)
    out = np.concatenate(
        [np.asarray(res.results[i]["out"], np.float32) for i in range(NCORES)],
        axis=0,
    )
    return out, res


def kernel(**inputs):
    out, _ = run(inputs, trace=False)
    return out


# revision 8
# speedup vs baseline: 1.5761x; 1.0265x over previous
"""VQ codebook kernel (nn_NaiveCodebook) for 8 TRN2 NeuronCores — bf16 v5.

Math (per batch row r):
    x   = (img1 - img2) @ W_in                      (b_in cancels in x1-x2)
    d2k = ||x||^2 - 2<x, b_k> + ||b_k||^2
    norm_res = sqrt(min_k d2k)
    scale = norm_res / ||rand|| + eps
    out = (x + scale * rand) @ W_out + b_out

All HBM streams are bf16, host pre-tiled so each DMA is 128 partitions
x one contiguous run.  Host-side work is layout only (transpose / cast /
tiling) plus constant-folding ||b_k||^2/2 from the codebook weights.

Device pipeline per core (rows = 512):
  A: stream img1^T/img2^T (interleaved) + W_in; diff on DVE; accumulate
     x^T = W_in^T @ diff^T into 4 PSUM banks (K=12288).  ||rand||^2 and
     its sqrt/reciprocal chain run here too (they have no deps).
  B: stream book^T; per 512-code tile: -c2/2 seeded into PSUM via a K=1
     ones (x) (-c2/2) matmul, then G accumulates; one DVE
     tensor_reduce(max) per tile into a column of m_all; one final
     reduce collapses the columns.
  S: ||x||^2 via ACT Square + ones-matmul partition sums; ns2 =
     -2*max + ||x||^2 (one fused op); scale = sqrt(ns2)/||rand|| + eps;
     broadcast via K=1 ones (x) scale matmul; q^T = x^T + s*r^T.
  C: stream W_out; paired 2-bank PSUM tiles; one DVE add (+bias bcast
     tile) per 1024 columns writes bf16 out tiles; DMA out.
"""

import os
import sys

for _p in (
    "/root/.axon_site",
    "/root/.axon_site/_ro/trn_rl_repo",
    "/opt/trn_rl_repo",
):
    if os.path.isdir(_p) and _p not in sys.path:
        sys.path.append(_p)

import numpy as np
import ml_dtypes

import concourse.bacc as bacc
import concourse.bass as bass
import concourse.tile as tile
from concourse import mybir
from concourse.bass_utils import run_bass_kernel_spmd

F32 = mybir.dt.float32
BF16 = mybir.dt.bfloat16
ALU = mybir.AluOpType
ACTF = mybir.ActivationFunctionType
BF = ml_dtypes.bfloat16

B, C_, H_, W_ = 4096, 3, 64, 64
IN_DIM = C_ * H_ * W_  # 12288
EMB = 512
K = 8192
EPS = 1e-6
NCORES = 8
P = 128
KB = 8


def build_program(rows=B // NCORES, in_dim=IN_DIM, emb=EMB, k=K, kb=KB):
    """Single-core Bass program (SPMD across 8 cores)."""
    assert rows % P == 0 and emb % P == 0 and in_dim % (P * kb) == 0
    assert k % 1024 == 0 and in_dim % 1024 == 0
    mch = rows // P           # row chunks (4)
    ech = emb // P            # emb chunks (4)
    nkb = in_dim // (P * kb)  # phase-A DMA batches (12)
    nd = k // 512             # codebook 512-tiles (16)
    ndp = nd // 2             # paired book DMAs (8)
    no = in_dim // 512        # output column tiles (24)
    nop = no // 2             # paired w_out DMAs / paired PSUM tiles (12)

    nc = bacc.Bacc()
    img12T = nc.declare_dram_parameter(
        "img12T", [nkb, P, kb, 2, rows], BF16, isOutput=False)
    w_in = nc.declare_dram_parameter(
        "w_in", [nkb, P, kb, emb], BF16, isOutput=False)
    bookT = nc.declare_dram_parameter(
        "bookT", [ndp, P, ech, 1024], BF16, isOutput=False)
    c2 = nc.declare_dram_parameter("c2", [1, k], F32, isOutput=False)
    randT = nc.declare_dram_parameter("randT", [P, ech, rows], BF16, isOutput=False)
    w_out = nc.declare_dram_parameter(
        "w_out", [nop, P, ech, 1024], BF16, isOutput=False)
    b_out = nc.declare_dram_parameter("b_out", [1, in_dim], BF16, isOutput=False)
    out = nc.declare_dram_parameter("out", [rows, in_dim], BF16, isOutput=True)

    def bcast_ap(handle, count):
        ap = handle.ap()
        return bass.AP(
            tensor=ap.tensor,
            offset=ap.offset,
            ap=[[0, count]] + list(ap.ap)[1:],
        )

    with tile.TileContext(nc) as tc:
        with tc.tile_pool(name="persist", bufs=1) as persist:
            xT = persist.tile([P, ech, rows], BF16, tag="xT")
            rT = persist.tile([P, ech, rows], BF16, tag="rT")
            qT = persist.tile([P, ech, rows], BF16, tag="qT")
            m_all = [
                persist.tile([P, nd], F32, tag=f"ma{m}", name=f"ma{m}")
                for m in range(mch)
            ]
            m_fin = [
                persist.tile([P, 1], F32, tag=f"mf{m}", name=f"mf{m}")
                for m in range(mch)
            ]
            ones_k = persist.tile([P, 1], BF16, tag="ones_k")   # K=128 sum lhsT
            ones_m = persist.tile([1, P], BF16, tag="ones_m")   # K=1 fold/bcast lhsT
            c2n = persist.tile([1, k], BF16, tag="c2n")         # -||b||^2/2
            bbb = persist.tile([P, in_dim], BF16, tag="bbb")    # bias bcast
            mT = persist.tile([1, rows], F32, tag="mT")
            ns2 = persist.tile([1, rows], F32, tag="ns2")
            nres = persist.tile([1, rows], F32, tag="nres")
            nrnd = persist.tile([1, rows], F32, tag="nrnd")
            nrir = persist.tile([1, rows], F32, tag="nrir")
            scT = persist.tile([1, rows], F32, tag="scT")
            scb = persist.tile([1, rows], BF16, tag="scb")
            sq = [
                persist.tile([P, rows], BF16, tag=f"sq{i}", name=f"sq{i}")
                for i in range(2)
            ]
            nc.vector.memset(ones_k, 1.0)
            nc.vector.memset(ones_m, 1.0)
            nc.gpsimd.dma_start(out=c2n, in_=c2.ap())   # f32 -> bf16 cast DMA
            nc.vector.tensor_scalar_mul(c2n, c2n, -1.0)
            nc.sync.dma_start(out=rT, in_=randT.ap())

            # ---------------- Phase A: xT = W_in^T @ diff^T ----------------
            with (
                tc.tile_pool(name="astream", bufs=2) as ast,
                tc.tile_pool(name="psum_a", bufs=1, space="PSUM") as psa,
            ):
                px = [
                    psa.tile([P, rows], F32, tag=f"px{e}", name=f"px{e}")
                    for e in range(ech)
                ]
                ps_nr = psa.tile([1, rows], F32, tag="pnr", name="ps_nr")
                # ||rand||^2 chain — no deps; hides under A's DMA-bound stretch
                for e in range(ech):
                    nc.scalar.activation(sq[e % 2], rT[:, e, :], ACTF.Square)
                    nc.tensor.matmul(
                        ps_nr, lhsT=ones_k, rhs=sq[e % 2],
                        start=(e == 0), stop=(e == ech - 1),
                    )
                nc.scalar.sqrt(nrnd, ps_nr[0:1, :])
                nc.vector.reciprocal(nrir, nrnd)
                for n in range(nkb):
                    t12 = ast.tile([P, kb, 2, rows], BF16, tag="t12")
                    wt = ast.tile([P, kb, emb], BF16, tag="wi")
                    dt = ast.tile([P, kb, rows], BF16, tag="dt")
                    nc.sync.dma_start(out=t12, in_=img12T.ap()[n])
                    nc.sync.dma_start(out=wt, in_=w_in.ap()[n])
                    nc.vector.tensor_sub(dt, t12[:, :, 0, :], t12[:, :, 1, :])
                    for j in range(kb):
                        for e in range(ech):
                            nc.tensor.matmul(
                                px[e],
                                lhsT=wt[:, j, e * P : (e + 1) * P],
                                rhs=dt[:, j, :],
                                start=(n == 0 and j == 0),
                                stop=(n == nkb - 1 and j == kb - 1),
                            )
                for e in range(ech):
                    if e % 2 == 0:
                        nc.vector.tensor_copy(xT[:, e, :], px[e])
                    else:
                        nc.scalar.copy(xT[:, e, :], px[e])

            # -------- Phase B: running max_k (G - c2/2) -------------------
            # cstream/couts open first so w_out prefetch + bbb land in SBUF
            # space disjoint from the book stream (no WAR on B's matmuls).
            outap = out.ap()
            with (
                tc.tile_pool(name="cstream", bufs=3) as cst,
                tc.tile_pool(name="couts", bufs=2) as cout,
            ):
                nc.sync.dma_start(out=bbb, in_=bcast_ap(b_out, P))
                with (
                    tc.tile_pool(name="bstream", bufs=3) as bst,
                    tc.tile_pool(name="bscratch", bufs=2) as bscr,
                    tc.tile_pool(name="psum_b", bufs=6, space="PSUM") as psb,
                    tc.tile_pool(name="psum_s", bufs=1, space="PSUM") as pss,
                ):
                    ps_sx = pss.tile([1, rows], F32, tag="psx", name="ps_sx")
                    ps_sc = pss.tile([P, rows], F32, tag="psc", name="ps_sc")
                    for bp in range(ndp):
                        bt = bst.tile([P, ech, 1024], BF16, tag="bt")
                        nc.sync.dma_start(out=bt, in_=bookT.ap()[bp])
                        for t in range(2):
                            n = bp * 2 + t
                            for m in range(mch):
                                ps = psb.tile([P, 512], F32, tag="d")
                                nc.tensor.matmul(
                                    ps,
                                    lhsT=ones_m,
                                    rhs=c2n[0:1, n * 512 : (n + 1) * 512],
                                    start=True,
                                    stop=False,
                                )
                                for e in range(ech):
                                    nc.tensor.matmul(
                                        ps,
                                        lhsT=xT[:, e, m * P : (m + 1) * P],
                                        rhs=bt[:, e, t * 512 : (t + 1) * 512],
                                        start=False,
                                        stop=(e == ech - 1),
                                    )
                                nc.vector.tensor_reduce(
                                    m_all[m][:, n : n + 1],
                                    ps,
                                    axis=mybir.AxisListType.X,
                                    op=ALU.max,
                                )
                    for m in range(mch):
                        nc.vector.tensor_reduce(
                            m_fin[m], m_all[m], axis=mybir.AxisListType.X,
                            op=ALU.max,
                        )

                    # ---------- Phase S: per-row scalars + quant^T ----------
                    for e in range(ech):
                        nc.scalar.activation(sq[e % 2], xT[:, e, :], ACTF.Square)
                        nc.tensor.matmul(
                            ps_sx, lhsT=ones_k, rhs=sq[e % 2],
                            start=(e == 0), stop=(e == ech - 1),
                        )
                    # mT[0, m*P + p] = m_fin[m][p]  (partition -> free)
                    for m in range(mch):
                        nc.sync.dma_start(
                            out=mT[0:1, m * P : (m + 1) * P], in_=m_fin[m]
                        )
                    # ns2 = ||x||^2 - 2*max = min_k d2; scale chain in bf16
                    nc.vector.scalar_tensor_tensor(
                        out=ns2, in0=mT, scalar=-2.0, in1=ps_sx[0:1, :],
                        op0=ALU.mult, op1=ALU.add,
                    )
                    nc.scalar.sqrt(nres, ns2)
                    nc.vector.tensor_mul(scT, nres, nrir)
                    nc.vector.tensor_scalar_add(scb, scT, EPS)
                    # broadcast: ps_sc = ones (x) scale
                    nc.tensor.matmul(
                        ps_sc, lhsT=ones_m, rhs=scb, start=True, stop=True
                    )
                    for e in range(ech):
                        tmp = bscr.tile([P, rows], F32, tag="tmp")
                        nc.vector.tensor_mul(tmp, rT[:, e, :], ps_sc)
                        nc.vector.tensor_add(qT[:, e, :], xT[:, e, :], tmp)

                # ---- Phase C: out = quant @ W_out + b_out ----
                with tc.tile_pool(name="psum_c", bufs=2, space="PSUM") as psc:
                    osb = None
                    for gg in range(nop):
                        wt = cst.tile([P, ech, 1024], BF16, tag="wo")
                        nc.sync.dma_start(out=wt, in_=w_out.ap()[gg])
                        if gg % 2 == 0:
                            osb = [
                                cout.tile([P, 2, 1024], BF16, tag=f"osb{m}", name=f"osb{m}")
                                for m in range(mch)
                            ]
                        for m in range(mch):
                            ps2 = psc.tile([P, 1024], F32, tag="o")
                            for t in range(2):
                                for e in range(ech):
                                    nc.tensor.matmul(
                                        ps2[:, t * 512 : (t + 1) * 512],
                                        lhsT=qT[:, e, m * P : (m + 1) * P],
                                        rhs=wt[:, e, t * 512 : (t + 1) * 512],
                                        start=(e == 0),
                                        stop=(e == ech - 1),
                                    )
                            nc.vector.tensor_add(
                                osb[m][:, gg % 2, :],
                                ps2,
                                bbb[:, gg * 1024 : (gg + 1) * 1024],
                            )
                        if gg % 2 == 1:
                            g = gg // 2
                            for m in range(mch):
                                nc.sync.dma_start(
                                    out=outap[
                                        m * P : (m + 1) * P,
                                        g * 2048 : (g + 1) * 2048,
                                    ],
                                    in_=osb[m],
                                )
    nc.finalize()
    return nc


def make_shards(image_1, image_2, random_vector, W_in, b_in, W_out, b_out, book,
                rows=None, ncores=NCORES, kb=KB):
    x1 = np.asarray(image_1, np.float32).reshape(np.shape(image_1)[0], -1)
    x2 = np.asarray(image_2, np.float32).reshape(np.shape(image_2)[0], -1)
    rv = np.asarray(random_vector, np.float32)
    nrows_total = x1.shape[0]
    if rows is None:
        rows = nrows_total // ncores
    in_dim = x1.shape[1]
    emb = np.shape(W_in)[1]
    k = np.shape(book)[0]
    ech = emb // P
    nkb = in_dim // (P * kb)
    ndp = k // 1024
    nop = in_dim // 1024

    x1b = x1.astype(BF)
    x2b = x2.astype(BF)
    w_in_b = np.asarray(W_in, np.float32).astype(BF)
    # w_in tiled: [nkb, P, kb, emb];  row index d = (n*kb + j)*P + p
    w_in_t = np.ascontiguousarray(
        w_in_b.reshape(nkb, kb, P, emb).transpose(0, 2, 1, 3)
    )
    bookT_b = np.asarray(book, np.float32).astype(BF).T  # [emb, k]
    bookT_t = np.ascontiguousarray(
        bookT_b.reshape(ech, P, ndp, 1024).transpose(2, 1, 0, 3)
    )
    c2_c = (np.sum(np.asarray(book, np.float64) ** 2, axis=1) / 2.0).astype(
        np.float32
    ).reshape(1, k)
    w_out_b = np.asarray(W_out, np.float32).astype(BF)  # [emb, in_dim]
    w_out_t = np.ascontiguousarray(
        w_out_b.reshape(ech, P, nop, 1024).transpose(2, 1, 0, 3)
    )
    b_out_c = np.ascontiguousarray(
        np.asarray(b_out, np.float32).astype(BF)
    ).reshape(1, in_dim)
    shards = []
    for i in range(ncores):
        sl = slice(i * rows, (i + 1) * rows)
        # img12 tiled: [nkb, P, kb, 2, rows]; d = (n*kb + j)*P + p
        img12 = np.empty((nkb, P, kb, 2, rows), BF)
        img12[:, :, :, 0, :] = (
            x1b[sl].T.reshape(nkb, kb, P, rows).transpose(0, 2, 1, 3)
        )
        img12[:, :, :, 1, :] = (
            x2b[sl].T.reshape(nkb, kb, P, rows).transpose(0, 2, 1, 3)
        )
        randT_t = np.ascontiguousarray(
            rv[sl].T.astype(BF).reshape(ech, P, rows).transpose(1, 0, 2)
        )
        shards.append(
            {
                "img12T": img12,
                "w_in": w_in_t,
                "bookT": bookT_t,
                "c2": c2_c,
                "randT": randT_t,
                "w_out": w_out_t,
                "b_out": b_out_c,
            }
        )
    return shards


_prog_cache = {}


def _get_program():
    if "nc" not in _prog_cache:
        _prog_cache["nc"] = build_program()
    return _prog_cache["nc"]


def run(inputs, trace=False):
    """Run on the 8 NeuronCores; returns (full_output, BassKernelResults)."""
    nc = _get_program()
    shards = make_shards(**inputs)
    res = run_bass_kernel_spmd(nc, shards, core_ids=list(range(NCORES)), trace=trace)
    out = np.concatenate(
        [np.asarray(res.results[i]["out"], np.float32) for i in range(NCORES)],
        axis=0,
    )
    return out, res


def kernel(**inputs):
    out, _ = run(inputs, trace=False)
    return out
